# revision 1
# baseline (speedup 1.0000x reference)
"""Trainium2 Bass kernel for BinsChamferLoss (multi-scale 1-D chamfer between
bin centers and depth-map pixels).

Problem shapes (hardcoded):
  bins:              [L=4, N=4, 257]  float32
  target_depth_maps: [N=4, 240, 320] float32  -> y: [N, M=76800]
  output: scalar float32 loss

Sharding: 16 (scale, batch) pairs over 8 cores -> each core handles one batch
n = core//2 and two scales {2*(core%2), 2*(core%2)+1}, scanning the full
76800-point set of its batch once.

Per-core device algorithm (points on partitions, centers on free dim):
  y_sb   [128, 600]   : 76800 points of batch n
  cb_sb  [128, 2,256] : the 2x256 bin centers, replicated across partitions
  bias   [128, 600]   : -y + 100*(y < eps)   (invalid points pushed far away)
  For each point-column j: d2[:, :, :] = Square(cb - y_j)  on ScalarE
  cham_y: segmented min over centers (VectorE tensor_reduce)
  cham_x: running elementwise min over point-columns (VectorE tensor_tensor)
Host combines tiny per-core partials (sums/mins over 128 lanes).
"""

import sys

if "/opt/trn_rl_repo" not in sys.path:
    sys.path.insert(0, "/opt/trn_rl_repo")

import numpy as np

EPS_DEPTH = 0.001
L, N, P1 = 4, 4, 257
P = P1 - 1            # 256 centers
M = 240 * 320         # 76800 points per batch
PARTS = 128
COLS = M // PARTS     # 600
J = 8                 # point-columns per inner block
NCORES = 8
SHIFT = 100.0         # pushes invalid points' distances to ~1e4
BIGF = 3.0e38

_cache = {}


def _build_module():
    import concourse.bacc as bacc
    import concourse.tile as tile
    from concourse import mybir

    nc = bacc.Bacc("TRN2", target_bir_lowering=False, debug=False)
    f32 = mybir.dt.float32

    y_d = nc.dram_tensor("y", [PARTS, COLS], f32, kind="ExternalInput").ap()
    cb_d = nc.dram_tensor("cb", [PARTS, 2, P], f32, kind="ExternalInput").ap()
    sumy_d = nc.dram_tensor("sumy", [PARTS, 2], f32, kind="ExternalOutput").ap()
    cnt_d = nc.dram_tensor("cnt", [PARTS, 1], f32, kind="ExternalOutput").ap()
    rminx_d = nc.dram_tensor("rminx", [PARTS, 2, P], f32, kind="ExternalOutput").ap()

    AF = mybir.ActivationFunctionType
    ALU = mybir.AluOpType
    AX = mybir.AxisListType

    with tile.TileContext(nc) as tc:
        with (
            tc.tile_pool(name="singles", bufs=1) as singles,
            tc.tile_pool(name="work", bufs=3) as work,
        ):
            y_sb = singles.tile([PARTS, COLS], f32)
            nc.sync.dma_start(out=y_sb, in_=y_d)
            cb_sb = singles.tile([PARTS, 2, P], f32)
            nc.sync.dma_start(out=cb_sb, in_=cb_d)

            mask = singles.tile([PARTS, COLS], f32)
            nc.vector.tensor_scalar(
                out=mask, in0=y_sb, scalar1=EPS_DEPTH, scalar2=None, op0=ALU.is_ge
            )
            # bias = 100*(y < eps) - y
            biasn = singles.tile([PARTS, COLS], f32)
            nc.vector.tensor_scalar(
                out=biasn, in0=y_sb, scalar1=EPS_DEPTH, scalar2=SHIFT,
                op0=ALU.is_lt, op1=ALU.mult,
            )
            nc.vector.tensor_sub(biasn, biasn, y_sb)

            miny = singles.tile([PARTS, COLS, 2], f32)
            rminx8 = singles.tile([PARTS, J, 2, P], f32)
            nc.gpsimd.memset(rminx8, BIGF)

            for base in range(0, COLS, J):
                d2 = work.tile([PARTS, J, 2, P], f32, tag="d2")
                for jj in range(J):
                    nc.scalar.activation(
                        d2[:, jj, :, :], cb_sb, AF.Square,
                        bias=biasn[:, base + jj : base + jj + 1], scale=1.0,
                    )
                # cham_y: per-point min over the 256 centers of each scale
                nc.vector.tensor_reduce(
                    out=miny[:, base : base + J, :], in_=d2, axis=AX.X, op=ALU.min
                )
                # cham_x: running elementwise min across point-columns
                nc.vector.tensor_tensor(
                    out=rminx8, in0=rminx8, in1=d2, op=ALU.min
                )

            # fold the J interleaved cham_x accumulators
            nc.vector.tensor_tensor(
                out=rminx8[:, 0:4, :, :], in0=rminx8[:, 0:4, :, :],
                in1=rminx8[:, 4:8, :, :], op=ALU.min,
            )
            nc.vector.tensor_tensor(
                out=rminx8[:, 0:2, :, :], in0=rminx8[:, 0:2, :, :],
                in1=rminx8[:, 2:4, :, :], op=ALU.min,
            )
            nc.vector.tensor_tensor(
                out=rminx8[:, 0:1, :, :], in0=rminx8[:, 0:1, :, :],
                in1=rminx8[:, 1:2, :, :], op=ALU.min,
            )
            nc.sync.dma_start(out=rminx_d, in_=rminx8[:, 0, :, :])

            # cham_y: mask invalid points, then per-lane per-scale sums
            sumy_sb = singles.tile([PARTS, 2], f32)
            for s in range(2):
                nc.vector.tensor_tensor(
                    out=miny[:, :, s], in0=miny[:, :, s], in1=mask, op=ALU.mult
                )
                nc.vector.tensor_reduce(
                    out=sumy_sb[:, s : s + 1], in_=miny[:, :, s], axis=AX.X,
                    op=ALU.add,
                )
            cnt_sb = singles.tile([PARTS, 1], f32)
            nc.vector.tensor_reduce(out=cnt_sb, in_=mask, axis=AX.X, op=ALU.add)
            nc.sync.dma_start(out=sumy_d, in_=sumy_sb)
            nc.sync.dma_start(out=cnt_d, in_=cnt_sb)

    nc.compile()
    return nc


def _get_module():
    if "nc" not in _cache:
        _cache["nc"] = _build_module()
    return _cache["nc"]


def kernel(bins: np.ndarray, target_depth_maps: np.ndarray) -> np.ndarray:
    from concourse.bass_utils import run_bass_kernel_spmd

    bins = np.asarray(bins, dtype=np.float32)
    maps = np.asarray(target_depth_maps, dtype=np.float32)

    centers = 0.5 * (bins[:, :, 1:] + bins[:, :, :-1])  # [L, N, 256] fp32

    in_maps = []
    for c in range(NCORES):
        n = c // 2
        s0 = 2 * (c % 2)
        y = np.ascontiguousarray(maps[n].reshape(PARTS, COLS))
        cb = np.ascontiguousarray(
            np.broadcast_to(centers[s0 : s0 + 2, n, :][None, :, :], (PARTS, 2, P))
        )
        in_maps.append({"y": y, "cb": cb})

    nc = _get_module()
    res = run_bass_kernel_spmd(nc, in_maps, core_ids=list(range(NCORES)))

    total = 0.0
    for c in range(NCORES):
        n = c // 2
        s0 = 2 * (c % 2)
        out = res.results[c]
        y_len = float(out["cnt"].astype(np.float64).sum())
        for s in range(2):
            cham_y = float(out["sumy"][:, s].astype(np.float64).sum()) / y_len
            cham_x = float(out["rminx"][:, s, :].min(axis=0).astype(np.float64).mean())
            total += (cham_x + cham_y) / N

    return np.float32(total)
